# revision 24
# baseline (speedup 1.0000x reference)
"""Density_loss (kNN k=16, B=8, N=2048, C=3) Trainium2 kernel.

Sharding: data-parallel over batch B=8 across 8 NeuronCores. Each core
handles one batch element of both `seed` and `gt_s`: it computes the full
NxN squared-distance matrix on the PE and extracts per-row top-k
candidates with the DVE; the final scalar MSE is assembled on host.

Device algorithm per row-tile [128, 2048]:
  PE:      -d[i,j] = sum_c lhsT[c,i]*rhs[c,j] with the augmented 5-dim
           factorization lhsT=[2x; -1; -|x|^2], rhs=[x; |x|^2; 1]
           -> PSUM (4 banks of 512), dtype float32r (1 cycle/row PE mode)
  ScalarE: copy PSUM -> SBUF (one [128,4,512] instruction)
  DVE:     max8 on each 512-wide quarter -> 32 candidates/row
           (~1.1 full passes instead of 3 for max8+match_replace+max8)

Host merge: top-16 of the 32 candidates per row. This equals the true
top-16 unless one quarter holds >= 9 of the true top-16; in that case
that quarter provably contributes all 8 of its candidates to the merged
top-16, so the condition "some quarter contributes >= 8" flags every
possibly-wrong row (~11% on randn data). Flagged rows are recomputed
exactly on host (cheap: ~2k rows x 2048 distances).
"""

import sys

import numpy as np

sys.path.insert(0, "/opt/trn_rl_repo")

import concourse.bacc as bacc  # noqa: E402
import concourse.bass as bass  # noqa: E402
import concourse.mybir as mybir  # noqa: E402
from concourse.bass_utils import run_bass_kernel_spmd  # noqa: E402
from concourse.tile import TileContext  # noqa: E402

B = 8
N = 2048
P = 128
NT = N // P  # 16 row-tiles per tensor
K = 16
Q = 4  # quarters (one PSUM bank each)
QW = N // Q  # 512
CPT = Q * 8  # candidates per row per tile = 32

_NC_CACHE = {}


CONTR = 24  # contraction rows after triple-bf16 expansion


def _build_nc(matmul_dtype=mybir.dt.bfloat16) -> bass.Bass:
    # Bacc (not plain Bass): its finalize() runs the wait-splitting passes
    # (move_matmul_waits_to_ldweights / generate_event_semaphores) required
    # to satisfy the TRN2 one-sync-wait-per-instruction constraint.
    nc = bacc.Bacc(
        "TRN2", target_bir_lowering=False, debug=False, num_devices=B
    )
    f32 = mybir.dt.float32

    # Packed input: [CONTR, 4*N] = (lhsT_seed | rhs_seed | lhsT_gt | rhs_gt)
    # along the free axis, so all four share base partition 0 (matmul
    # requires lhsT and rhs to live on the same base partition).
    inp = nc.declare_dram_parameter(
        "inp", [CONTR, 4 * N], matmul_dtype, isOutput=False
    )
    out = nc.declare_dram_parameter("out", [P, 2 * NT * CPT], f32, isOutput=True)

    with TileContext(nc) as tc:
        with (
            tc.tile_pool(name="inputs", bufs=1) as ipool,
            tc.tile_pool(name="slab", bufs=1) as spool,
            tc.tile_pool(name="psum", bufs=2, space="PSUM") as ppool,
            tc.tile_pool(name="work", bufs=4) as wpool,
        ):
            it = ipool.tile([CONTR, 4 * N], matmul_dtype, tag="inp")
            # Two DMAs so seed compute starts before the gt half lands
            # (HWDGE descriptor generation is ~625ns per DMA, so fewer,
            # larger DMAs reach the first matmul sooner).
            nc.sync.dma_start(out=it[:, : 2 * N], in_=inp[:, : 2 * N])
            nc.sync.dma_start(out=it[:, 2 * N :], in_=inp[:, 2 * N :])
            li_s = it[:, 0 * N : 1 * N]
            ri_s = it[:, 1 * N : 2 * N]
            li_g = it[:, 2 * N : 3 * N]
            ri_g = it[:, 3 * N : 4 * N]

            slab = spool.tile([P, 2 * NT * CPT], f32, tag="slab")

            for tid, (lt, rt) in enumerate(((li_s, ri_s), (li_g, ri_g))):
                for t in range(NT):
                    pt = ppool.tile([P, Q, QW], f32, tag="pt")
                    for q in range(Q):
                        nc.tensor.matmul(
                            pt[:, q, :],
                            lt[:, t * P : (t + 1) * P],
                            rt[:, q * QW : (q + 1) * QW],
                            start=True,
                            stop=True,
                        )
                    d = wpool.tile([P, N], f32, tag="d")
                    col = (tid * NT + t) * CPT
                    if tid == 0 and t == 0:
                        # Prologue tile: per-bank copy so the first max8
                        # fires as soon as bank 0 is copied, shrinking the
                        # DVE fill stall.
                        copies = [(h, 1) for h in range(Q)]
                    else:
                        # Steady state: two half-copies (less ScalarE
                        # per-op overhead; DVE is the bottleneck).
                        copies = [(0, 2), (2, 2)]
                    for h, w in copies:
                        nc.scalar.copy(
                            out=d[:, h * QW : (h + w) * QW].rearrange(
                                "p (a b) -> p a b", a=w
                            ),
                            in_=pt[:, h : h + w, :],
                        )
                    for q in range(Q):
                        nc.vector.max(
                            out=slab[:, col + q * 8 : col + q * 8 + 8],
                            in_=d[:, q * QW : (q + 1) * QW],
                        )
                # Write back this tensor's slab in two chunks; all but the
                # last overlap with remaining compute.
                for h in range(2):
                    base = (2 * tid + h) * (NT // 2) * CPT
                    nc.sync.dma_start(
                        out=out[:, base : base + (NT // 2) * CPT],
                        in_=slab[:, base : base + (NT // 2) * CPT],
                    )

    nc.finalize()
    return nc


def _split3(v: np.ndarray):
    """Exact-ish triple-bf16 split: v ~= vh + vm + vl (f32 views)."""
    import ml_dtypes

    bf = ml_dtypes.bfloat16
    vh = v.astype(bf).astype(np.float32)
    r = v - vh
    vm = r.astype(bf).astype(np.float32)
    vl = (r - vm).astype(bf)
    return vh.astype(bf), vm.astype(bf), vl


def _prep(x: np.ndarray):
    """x: [N, 3] f32 -> (lhsT [24,N], rhs [24,N]) bf16 so that
    (lhsT.T @ rhs)[i, j] ~= -||x_i - x_j||^2 to ~f32 accuracy.

    Each f32 factor is split into hi/mid/lo bf16 components; per
    coordinate the 6 dominant cross products (hh, hm, mh, hl, lh, mm)
    are kept, dropping only O(2^-27)-relative terms. The |x|^2 columns
    are paired against exact +-1 so their split is lossless."""
    import ml_dtypes

    bf = ml_dtypes.bfloat16
    x = np.ascontiguousarray(x, dtype=np.float32)
    n = x.shape[0]
    sq = (x * x).sum(axis=1, dtype=np.float32)
    ones = np.ones(n, dtype=bf)

    lrows, rrows = [], []
    for c in range(3):
        ah, am, al = _split3(2.0 * x[:, c])
        bh, bm, bl = _split3(x[:, c])
        lrows += [ah, ah, am, ah, al, am]
        rrows += [bh, bm, bh, bl, bh, bm]
    sh, sm, sl = _split3(sq)
    lrows += [-ones, -ones, -ones, -sh, -sm, -sl]
    rrows += [sh, sm, sl, ones, ones, ones]

    lhsT = np.ascontiguousarray(np.stack(lrows))
    rhs = np.ascontiguousarray(np.stack(rrows))
    assert lhsT.shape == (CONTR, n) and lhsT.dtype == bf
    return lhsT, rhs


def _get_nc():
    if "nc" not in _NC_CACHE:
        _NC_CACHE["nc"] = _build_nc()
    return _NC_CACHE["nc"]


def _topk_sums_from_slab(half: np.ndarray, x: np.ndarray) -> float:
    """half: [128, NT*CPT] device candidates (values are -d, top-8 per
    512-quarter, descending). x: [N, 3] raw points for exact host fix-up.
    Returns sum over all rows of (sum of 16 largest -d values)."""
    # [128, NT, Q, 8] -> rows: row = t*128 + p
    c = half.reshape(P, NT, Q, 8).transpose(1, 0, 2, 3).reshape(N, Q, 8)
    flat = c.reshape(N, Q * 8)
    # top-16 of the 32 candidates, descending
    part = -np.partition(-flat, K - 1, axis=1)[:, :K]
    thr = np.min(part, axis=1)  # 16th largest candidate value
    contrib = (c >= thr[:, None, None]).sum(axis=2)  # [N, Q]
    suspect = (contrib >= 8).any(axis=1)
    sums = part.sum(axis=1, dtype=np.float64)

    if suspect.any():
        idx = np.nonzero(suspect)[0]
        xs = np.ascontiguousarray(x, dtype=np.float32)
        sq = (xs * xs).sum(1, dtype=np.float32)
        # -d rows for suspect points, exact top-16 (f32 products match
        # device precision; sums accumulate in f64)
        rows = 2.0 * (xs[idx] @ xs.T) - sq[None, :] - sq[idx, None]
        top = -np.partition(-rows, K - 1, axis=1)[:, :K]
        sums[idx] = top.sum(axis=1, dtype=np.float64)
    return float(sums.sum())


def kernel(seed: np.ndarray, gt_s: np.ndarray) -> np.ndarray:
    seed = np.asarray(seed, dtype=np.float32)
    gt_s = np.asarray(gt_s, dtype=np.float32)
    assert seed.shape == (B, N, 3) and gt_s.shape == (B, N, 3)

    nc = _get_nc()
    in_maps = []
    for b in range(B):
        ls, rs = _prep(seed[b])
        lg, rg = _prep(gt_s[b])
        in_maps.append({"inp": np.concatenate([ls, rs, lg, rg], axis=1)})

    res = run_bass_kernel_spmd(nc, in_maps, list(range(B))).results

    dis = np.empty(B, dtype=np.float64)
    gt = np.empty(B, dtype=np.float64)
    scale = 1.0 / (N * K)
    for b in range(B):
        slab = res[b]["out"]  # [128, 2*NT*CPT]; values are -d candidates
        dis[b] = -_topk_sums_from_slab(slab[:, : NT * CPT], seed[b]) * scale
        gt[b] = -_topk_sums_from_slab(slab[:, NT * CPT :], gt_s[b]) * scale

    val = np.mean((dis - gt) ** 2)
    return np.array(val, dtype=np.float32)
